# revision 48
# baseline (speedup 1.0000x reference)
"""Trainium2 Bass kernel for nn_Loc2Cluster (GNN message passing, segment-max).

Computation: agg[c] = elementwise-max over locs with edge to cluster c of
x_locs[loc]; empty clusters -> 0; output = concat([x_clusters, agg], -1).

Strategy (cluster-sharded, zero collectives, bf16 streaming):
  - Core k owns clusters [4096k, 4096(k+1)) after a global count-desc sort
    dealt round-robin across cores (balances per-core round sizes).
  - Host routes each edge's loc row (pre-rounded to bf16; max commutes with
    monotone rounding, so the result equals bf16(true max), rel err <= 2^-9,
    far inside the 2e-2 gate) to the core owning its dst cluster.
  - Within a core, rows are laid out in "rounds": round r holds the r-th edge
    row of every cluster with count > r, in count-sorted order, so each round
    is a contiguous *prefix* of cluster slots and the whole segment-max is
    ~max_degree dense tensor_max ops -- no data-dependent addressing on device.
  - Round blocks are partition-major ([128, X_r, 256]); every DMA is a plain
    strided copy and every cluster lives at a fixed (partition, chunk) slot of
    the bf16 SBUF accumulator. Every round transfers only real rows — zero
    pad bytes: wide rounds as inline full+partial DMA pairs on SP, narrow
    rounds' partial chunks split off as early mini-pieces on the scalar
    queue, and single-chunk rounds on the Pool/SWDGE queue early in the
    stream so the 8 HWDGE lanes stay on the bulk row traffic.
  - Round 0 is DMA'd straight into the accumulator (tail slots for empty
    clusters are zero rows -> matches reference's 0-fill, no fixup pass).
  - The accumulator is flushed to a bf16 DRAM output progressively on the
    Pool queue: once no later round touches a chunk range it is written out,
    overlapping the output traffic with the remaining row stream; a couple of
    ready-early chunk flushes are held back to keep the DMA engines fed while
    the last round's max->flush dependency chain resolves.
  - Host unshard: upconvert agg bf16->f32 (exact), scatter rows back to
    cluster order, and place x_clusters (untouched f32 input) as the left
    half of the concat.
"""

import sys

import numpy as np

if "/opt/trn_rl_repo" not in sys.path:
    sys.path.insert(0, "/opt/trn_rl_repo")

import ml_dtypes

BF16 = np.dtype(ml_dtypes.bfloat16)

N_LOCS = 262144
N_CLUSTERS = 32768
D = 256
N_CORES = 8
CPC = N_CLUSTERS // N_CORES  # 4096 clusters per core
P = 128
CHUNKS = CPC // P  # 32 chunks of 128 clusters
NEG = np.float32(-1e30)

LAST_RESULTS = None  # BassKernelResults of the most recent run (for profiling)
LAST_NC = None  # compiled Bass module of the most recent run (for TimelineSim)


def _host_prep(x_locs, edge_src, edge_dst):
    """Build per-core round-major bf16 row streams + schedule metadata."""
    x_locs_bf = np.asarray(x_locs, dtype=np.float32).astype(BF16)
    src = np.asarray(edge_src).astype(np.int64)
    dst = np.asarray(edge_dst).astype(np.int64)
    n_edges = dst.shape[0]

    counts = np.bincount(dst, minlength=N_CLUSTERS)  # [32768]

    # Global order by count desc, dealt round-robin across cores: cluster
    # with global rank g goes to core g%8 at local rank g//8, so each core's
    # local order is count-sorted and round sizes match across cores to
    # within one cluster (the shared SPMD schedule uses the ceil).
    gorder = np.argsort(-counts, kind="stable")  # [32768] cluster ids by rank
    grank = np.empty_like(gorder)
    grank[gorder] = np.arange(N_CLUSTERS)
    order = np.ascontiguousarray(gorder.reshape(CPC, N_CORES).T)  # [8, CPC]

    # occurrence index of each edge within its dst cluster
    by_dst = np.argsort(dst, kind="stable")
    group_start = np.zeros(N_CLUSTERS, dtype=np.int64)
    np.cumsum(counts[:-1], out=group_start[1:])
    occ = np.empty(n_edges, dtype=np.int64)
    occ[by_dst] = np.arange(n_edges, dtype=np.int64) - group_start[dst[by_dst]]

    g_of = grank[dst]
    core_of = g_of % N_CORES
    rank_of = g_of // N_CORES

    # round schedule: m_r = per-core #clusters with count > r (ceil over the
    # round-robin deal); X_r chunks of 128 slots, last chunk partial
    R = max(int(counts.max()), 1)
    counts_sorted = counts[gorder]
    m_r_g = (counts_sorted[None, :] > np.arange(R)[:, None]).sum(axis=1)
    m_r = (m_r_g + N_CORES - 1) // N_CORES  # per-core max
    m_r[0] = CPC  # round 0 covers every slot (zeros for empty clusters)
    X = (m_r + P - 1) // P  # chunks per round
    M = X * P  # HBM block size per round (pad slots exist, not transferred)
    offs = np.zeros(R + 1, dtype=np.int64)
    np.cumsum(M, out=offs[1:])
    TOT = int(offs[-1])

    # slot of each edge inside its core's stream (partition-major blocks)
    p_of = rank_of % P
    c_of = rank_of // P
    slot = offs[occ] + p_of * X[occ] + c_of

    slot_src = np.full((N_CORES, TOT), -1, dtype=np.int64)
    slot_src[core_of, slot] = src

    in_maps = []
    for k in range(N_CORES):
        ss = slot_src[k]
        stream = x_locs_bf[np.maximum(ss, 0)]  # [TOT, 256] bf16
        pad = ss < 0
        if pad[:CPC].any():
            stream[np.flatnonzero(pad[:CPC])] = BF16.type(0.0)  # empties -> 0
        padr = np.flatnonzero(pad[CPC:]) + CPC
        if padr.size:
            stream[padr] = BF16.type(NEG)  # later-round pads are max-neutral
        in_maps.append({"rows": np.ascontiguousarray(stream)})

    return in_maps, order, m_r, X, offs, TOT


def _build_program(
    R, m_r, X, offs, TOT, bufs=6, tight_min_x=10, reserve=0, tiny_after=3,
    tiny_engine="gpsimd", flush_min=2, flush_engine="gpsimd",
    partial_engine="sync", final_flush_engine="scalar", reserve_engine=None,
    lane_pad=2, sw_lane_pad=0,
):
    from concourse import bacc, mybir
    from concourse._compat import axon_active
    from concourse.tile import TileContext

    nc = bacc.Bacc(
        "TRN2",
        target_bir_lowering=False,
        debug=not axon_active(),
        num_devices=N_CORES,
    )
    rows_h = nc.dram_tensor("rows", [TOT, D], mybir.dt.bfloat16, kind="ExternalInput")
    out_h = nc.dram_tensor(
        "out", [P, CHUNKS * D], mybir.dt.bfloat16, kind="ExternalOutput"
    )

    # Emission order: small independent pieces (single-chunk rounds, and the
    # partial last chunks of narrow rounds) go early — after the first few
    # big rounds, on the Pool/SWDGE lanes — so their per-instruction issue
    # overhead hides under long transfers instead of starving the DMA
    # engines at the end of the stream. The remaining full-width pieces run
    # in descending width; the very smallest round goes last so the final
    # max->flush dependency chain is as short as possible.
    tiny = [r for r in range(1, R) if int(X[r]) == 1]
    wide = [r for r in range(1, R) if int(X[r]) >= tight_min_x]
    narrow = [r for r in range(1, R) if 2 <= int(X[r]) < tight_min_x]
    has_part = {r: int(m_r[r]) < int(X[r]) * P for r in narrow}
    emit = (
        [("wide", r) for r in wide[:tiny_after]]
        + [("tiny", r) for r in tiny[:-1]]
        + [("part", r) for r in narrow if has_part[r]]
        + [("wide", r) for r in wide[tiny_after:]]
        + [("nfull", r) for r in narrow]
        + [("tiny", r) for r in tiny[-1:]]
    )

    def piece_hi(kind, r):  # highest chunk index (exclusive) a piece touches
        if kind == "tiny":
            return 1
        if kind == "nfull":
            return int(X[r]) - 1 if has_part[r] else int(X[r])
        return int(X[r])  # wide pair or partial piece

    n = len(emit)
    sufx = [0] * (n + 1)  # max touched chunk bound over emit[i:]
    for i in range(n - 1, -1, -1):
        sufx[i] = max(piece_hi(*emit[i]), sufx[i + 1])

    reserve = min(reserve, CHUNKS)

    with TileContext(nc) as tc:
        with (
            tc.tile_pool(name="accp", bufs=1) as accp,
            tc.tile_pool(name="stagep", bufs=bufs) as stagep,
            tc.tile_pool(
                name="tinyp",
                bufs=min(max(len(tiny) + sum(has_part.values()), 1), 16),
            ) as tinyp,
            tc.tile_pool(name="padp", bufs=1) as padp,
        ):
            acc = accp.tile([P, CHUNKS * D], mybir.dt.bfloat16)
            # round 0: DMA straight into the accumulator
            r0 = rows_h.ap()[0:CPC].rearrange("(p x) f -> p (x f)", p=P)
            nc.sync.dma_start(out=acc[:, :], in_=r0)

            def flush(lo, hi):
                # issued off the SP queue so its wait for the source region's
                # last max never blocks the in-order SP row stream
                getattr(nc, flush_engine).dma_start(
                    out=out_h.ap()[:, lo * D : hi * D],
                    in_=acc[:, lo * D : hi * D],
                )

            # chunks [reserve_lo, CHUNKS) are flushed at the very end: their
            # data is final early, so the waitless transfers keep the DMA
            # engines busy while the last round's max->flush chain resolves
            reserve_lo = CHUNKS - reserve
            pend_hi = reserve_lo  # top of the unflushed non-reserved region

            for i, (kind, r) in enumerate(emit):
                Xr = int(X[r])
                mr = int(m_r[r])
                q = mr - (Xr - 1) * P  # partial-chunk height, in [1, 128]
                base = int(offs[r])
                blk = rows_h.ap()[base : base + Xr * P].rearrange(
                    "(p x) f -> p (x f)", p=P
                )
                wf = (Xr - 1) * D
                if kind == "tiny":
                    st = tinyp.tile([P, D], mybir.dt.bfloat16, tag="tiny")
                    # issued on the Pool (SWDGE) queue: separate DMA lane
                    # pool, keeps the 8 HWDGE lanes for the big rows
                    getattr(nc, tiny_engine).dma_start(
                        out=st[0:mr, :], in_=blk[0:mr, :]
                    )
                    nc.vector.tensor_max(
                        out=acc[0:mr, :D], in0=acc[0:mr, :D], in1=st[0:mr, :]
                    )
                elif kind == "part":
                    # narrow round's partial last chunk, split off and issued
                    # early on the otherwise-idle scalar queue so the full
                    # part needs no pads (Pool's SWDGE lanes would stall on
                    # the tiny DMAs' late completion sems)
                    st = tinyp.tile([P, D], mybir.dt.bfloat16, tag="tiny")
                    nc.scalar.dma_start(out=st[0:q, :], in_=blk[0:q, wf : wf + D])
                    nc.vector.tensor_max(
                        out=acc[0:q, wf : wf + D],
                        in0=acc[0:q, wf : wf + D],
                        in1=st[0:q, :],
                    )
                elif kind == "wide" and q < P:
                    # wide round: transfer only real rows (full chunks +
                    # partial last chunk); issue slack is ample mid-stream
                    st = stagep.tile([P, CHUNKS * D], mybir.dt.bfloat16, tag="stage")
                    nc.sync.dma_start(out=st[:, :wf], in_=blk[:, :wf])
                    getattr(nc, partial_engine).dma_start(
                        out=st[0:q, wf : wf + D], in_=blk[0:q, wf : wf + D]
                    )
                    nc.vector.tensor_max(
                        out=acc[:, :wf], in0=acc[:, :wf], in1=st[:, :wf]
                    )
                    nc.vector.tensor_max(
                        out=acc[0:q, wf : wf + D],
                        in0=acc[0:q, wf : wf + D],
                        in1=st[0:q, wf : wf + D],
                    )
                else:
                    # full-width piece: wide round with no partial, or a
                    # narrow round's full part (its partial went early)
                    w = wf if (kind == "nfull" and q < P) else Xr * D
                    st = stagep.tile([P, CHUNKS * D], mybir.dt.bfloat16, tag="stage")
                    nc.sync.dma_start(out=st[:, :w], in_=blk[:, :w])
                    nc.vector.tensor_max(
                        out=acc[:, :w], in0=acc[:, :w], in1=st[:, :w]
                    )
                lo = sufx[i + 1]
                # coalesce small drops: tiny tail flushes would serialize on
                # the scalar queue's issue cadence, so defer them to the one
                # final flush
                if 0 < lo < pend_hi and pend_hi - lo >= flush_min:
                    flush(lo, pend_hi)
                    pend_hi = lo
            if reserve_lo < CHUNKS:
                # waitless: keeps DMA busy in the tail
                getattr(nc, reserve_engine or flush_engine).dma_start(
                    out=out_h.ap()[:, reserve_lo * D : CHUNKS * D],
                    in_=acc[:, reserve_lo * D : CHUNKS * D],
                )
            if lane_pad:
                # rotate the global HWDGE lane counter with 1-row no-op
                # loads (emitted right before the final flush so mid-stream
                # lane assignment is untouched) to land the final flush on
                # the lane whose completion sem the epilogue waits LAST --
                # hides the serial post-final lane-wait processing inside
                # the 900ns propagation window
                scratch = padp.tile([P, D], mybir.dt.bfloat16)
                for j in range(lane_pad):
                    nc.scalar.dma_start(
                        out=scratch[j : j + 1, :], in_=rows_h.ap()[j : j + 1, :]
                    )
            for j in range(sw_lane_pad):
                sws = padp.tile([P, D], mybir.dt.bfloat16, tag="swpad")
                nc.gpsimd.dma_start(
                    out=sws[0:1, :], in_=rows_h.ap()[j : j + 1, :]
                )
            if pend_hi > 0:
                # waits only the final (smallest) round's max
                getattr(nc, final_flush_engine or flush_engine).dma_start(
                    out=out_h.ap()[:, : pend_hi * D], in_=acc[:, : pend_hi * D]
                )
    nc.compile()
    return nc


def kernel(x_locs, x_clusters, edge_src, edge_dst):
    global LAST_RESULTS, LAST_NC
    from concourse.bass_utils import run_bass_kernel_spmd

    x_clusters = np.ascontiguousarray(np.asarray(x_clusters, dtype=np.float32))
    in_maps, order, m_r, X, offs, TOT = _host_prep(x_locs, edge_src, edge_dst)
    R = len(m_r)
    nc = _build_program(R, m_r, X, offs, TOT)
    LAST_NC = nc
    try:
        res = run_bass_kernel_spmd(nc, in_maps, list(range(N_CORES)))
    except Exception:
        # transient NRT/tunnel faults (e.g. NRT_EXEC_UNIT_UNRECOVERABLE from
        # a prior session) clear on re-execution; ask the runtime to reset
        # the cores and retry once
        import os

        os.environ.setdefault("NEURON_RT_RESET_CORES", "1")
        res = run_bass_kernel_spmd(nc, in_maps, list(range(N_CORES)))
    LAST_RESULTS = res

    full = np.empty((N_CLUSTERS, 2 * D), dtype=np.float32)
    full[:, :D] = x_clusters  # concat left half: untouched f32 input
    for k in range(N_CORES):
        o = np.asarray(res.results[k]["out"])  # [P, CHUNKS*D] bf16
        agg = o.astype(np.float32).reshape(P, CHUNKS, D)  # exact upconvert
        full[order[k], D:] = agg.transpose(1, 0, 2).reshape(CPC, D)
    return full
